# revision 15
# baseline (speedup 1.0000x reference)
"""Trainium2 Bass kernel for nn_Attention (CBAM-style channel+spatial attention).

Computes, for x [4, 32, 64, 64, 64]:
  ca[b, c]       = sigmoid(MLP(concat(mean_dhw(x), max_dhw(x))))
  sa[b, d, h, w] = sigmoid(conv2(relu(conv1(concat(mean_c(x), max_c(x))))))
  attention      = sa * ca;  anti_attention = 1 - attention

Sharded over 8 NeuronCores as (batch, D-half); each core gets a host-padded
40-plane slab (4 halo planes each side) pre-rearranged into the on-chip
layout.  Cross-core traffic is one AllGather of 64 stats floats.
"""
import numpy as np
import ml_dtypes

BF16 = ml_dtypes.bfloat16

B, C, D, H, W = 4, 32, 64, 64, 64
K = 7
NCORES = 8
HALO = 4
DL = 40            # local planes per core (32 own + 2*4 halo)
NCHUNK = 5         # 8-plane chunks
CP = 8             # planes per chunk
PFC = CP * 32      # f-cols per channel per chunk (d_loc*32 + h//2) = 256
HP = H + 6         # padded h extent in s_conv (70)
NVOX = float(D * H * W)

_CACHE = {}


def _build_nc():
    import concourse.bacc as bacc
    import concourse.mybir as mybir
    from concourse import tile

    f32 = mybir.dt.float32
    bf16 = mybir.dt.bfloat16
    Alu = mybir.AluOpType
    Act = mybir.ActivationFunctionType
    Ax = mybir.AxisListType

    nc = bacc.Bacc("TRN2", target_bir_lowering=False, debug=False,
                   num_devices=NCORES)

    # ---- external I/O ----
    x_ext = nc.declare_dram_parameter("x", [NCHUNK, 128, 32 * PFC], bf16, isOutput=False)
    convw_ext = nc.declare_dram_parameter("convw", [128, 98 * 128], bf16, isOutput=False)
    oh_ext = nc.declare_dram_parameter("oh", [128, 32 * 32], bf16, isOutput=False)
    id_ext = nc.declare_dram_parameter("ident", [128, 128], f32, isOutput=False)
    idb_ext = nc.declare_dram_parameter("identb", [128, 128], bf16, isOutput=False)
    c2_ext = nc.declare_dram_parameter("c2w", [128, 128], bf16, isOutput=False)
    fc1w_ext = nc.declare_dram_parameter("fc1w", [128, 64], f32, isOutput=False)
    fc1b_ext = nc.declare_dram_parameter("fc1b", [128, 1], f32, isOutput=False)
    fc2w_ext = nc.declare_dram_parameter("fc2w", [32, 128], f32, isOutput=False)
    fc2b_ext = nc.declare_dram_parameter("fc2b", [32, 1], f32, isOutput=False)
    mask_ext = nc.declare_dram_parameter("masks", [16, 2], f32, isOutput=False)
    attn_ext = nc.declare_dram_parameter("attn", [4, 8, 128, 1024], f32, isOutput=True)
    anti_ext = nc.declare_dram_parameter("anti", [4, 8, 128, 1024], f32, isOutput=True)

    cc_in = nc.dram_tensor("cc_in", [2, 32], f32)
    cc_out = nc.dram_tensor("cc_out", [16, 32], f32, addr_space="Shared")

    with tile.TileContext(nc) as tc:
        with (
            tc.tile_pool(name="consts", bufs=1) as consts,
            tc.tile_pool(name="xpool", bufs=2) as xpool,
            tc.tile_pool(name="sconv", bufs=1) as sconvp,
            tc.tile_pool(name="small", bufs=2) as small,
            tc.tile_pool(name="tree", bufs=1) as treep,
            tc.tile_pool(name="shift", bufs=2) as shiftp,
            tc.tile_pool(name="relu", bufs=2) as relup,
            tc.tile_pool(name="saw", bufs=2) as sawp,
            tc.tile_pool(name="stat", bufs=1) as statp,
            tc.tile_pool(name="outp", bufs=2) as outp,
            tc.tile_pool(name="pcs", bufs=1, space="PSUM") as pcsp,
            tc.tile_pool(name="psp", bufs=1, space="PSUM") as pspp,
            tc.tile_pool(name="pconv", bufs=4, space="PSUM") as pconvp,
            tc.tile_pool(name="ptp", bufs=1, space="PSUM") as ptpp,
            tc.tile_pool(name="pmisc", bufs=1, space="PSUM") as pmiscp,
        ):
            # ---- constants ----
            oh = consts.tile([128, 32 * 32], bf16)
            nc.gpsimd.dma_start(oh[:], oh_ext[:])
            ident = consts.tile([128, 128], f32)
            nc.gpsimd.dma_start(ident[:], id_ext[:])
            identb = consts.tile([128, 128], bf16)
            nc.gpsimd.dma_start(identb[:], idb_ext[:])
            c2w = consts.tile([128, 128], bf16)
            nc.gpsimd.dma_start(c2w[:], c2_ext[:])
            fc1w = consts.tile([128, 64], f32)
            nc.gpsimd.dma_start(fc1w[:], fc1w_ext[:])
            fc1b = consts.tile([128, 1], f32)
            nc.gpsimd.dma_start(fc1b[:], fc1b_ext[:])
            fc2w = consts.tile([32, 128], f32)
            nc.gpsimd.dma_start(fc2w[:], fc2w_ext[:])
            fc2b = consts.tile([32, 1], f32)
            nc.gpsimd.dma_start(fc2b[:], fc2b_ext[:])
            masks = consts.tile([16, 2], f32)
            nc.gpsimd.dma_start(masks[:], mask_ext[:])
            convw = consts.tile([128, 98 * 128], bf16)
            nc.gpsimd.dma_start(convw[:], convw_ext[:])
            ones1 = consts.tile([1, 128], f32)
            nc.vector.memset(ones1[:], 1.0)

            # warm the ACT sigmoid/relu table set off the critical path
            warm = consts.tile([1, 1], f32)
            nc.vector.memset(warm[:], 0.0)
            warm2 = consts.tile([1, 1], f32)
            nc.scalar.activation(warm2[:], warm[:], Act.Sigmoid)

            # persistent accumulators / results
            s_conv = sconvp.tile([128, DL * HP], bf16)       # rows: i*64+w; f: d*70+3+h
            nc.vector.memset(s_conv[:], 0.0)
            spmax_parts = statp.tile([128, 32 * NCHUNK], f32)
            sa128 = statp.tile([128, 1024], f32)            # p=(do%2)*64+h, f=(do//2)*64+w
            ca_rep = statp.tile([128, 32], f32)
            nca_rep = statp.tile([128, 32], f32)
            psum_sp = pspp.tile([32, 256], f32)             # per-channel spatial sums

            relu_tiles = [[None, None] for _ in range(4)]
            sp_first = [True]

            def stage1_chunk(k):
                x_k = xpool.tile([128, 32 * PFC], bf16, tag="xk")
                eng = nc.sync if k % 2 == 0 else nc.scalar
                eng.dma_start(x_k[:], x_ext[k])

                # channel-sum (identity-matmul accumulation over the 32 channels)
                pcs = pcsp.tile([128, PFC], f32, tag="pcs")
                for c in range(32):
                    nc.tensor.matmul(pcs[:], identb[:], x_k[:, c * PFC:(c + 1) * PFC],
                                     start=(c == 0), stop=(c == 31),
                                     skip_group_check=True)

                # per-channel spatial sums over own planes -> psum_sp (accumulates)
                off, end = (128, 256) if k == 0 else ((0, 128) if k == 4 else (0, 256))
                n = end - off
                for c in range(32):
                    nc.tensor.matmul(psum_sp[:, off:end], oh[:, c * 32:(c + 1) * 32],
                                     x_k[:, c * PFC + off: c * PFC + end],
                                     start=sp_first[0],
                                     stop=(k == 4 and c == 31),
                                     skip_group_check=True)
                    sp_first[0] = False

                # per-channel spatial max over own planes: one strided reduce
                # (emitted first: the ca stats gate the whole output phase)
                nc.vector.tensor_reduce(
                    spmax_parts[:, k * 32:(k + 1) * 32],
                    x_k[:].rearrange("p (c f) -> p c f", c=32)[:, :, off:end],
                    axis=Ax.X, op=Alu.max)

                # channel-max: binary tensor_max tree (bf16 runs at 2x mode)
                t1 = treep.tile([128, 4096], bf16, tag="tr1")
                t2 = treep.tile([128, 2048], bf16, tag="tr2")
                t3 = treep.tile([128, 1024], bf16, tag="tr3")
                t4 = treep.tile([128, 512], bf16, tag="tr4")
                cmx = small.tile([128, PFC], bf16, tag="cmx")
                xv = x_k[:].rearrange("p (c f) -> p c f", c=32)
                nc.vector.tensor_max(t1[:].rearrange("p (c f) -> p c f", c=16),
                                     xv[:, 0:32:2, :], xv[:, 1:32:2, :])
                v1 = t1[:].rearrange("p (c f) -> p c f", c=16)
                nc.vector.tensor_max(t2[:].rearrange("p (c f) -> p c f", c=8),
                                     v1[:, 0:16:2, :], v1[:, 1:16:2, :])
                v2 = t2[:].rearrange("p (c f) -> p c f", c=8)
                nc.vector.tensor_max(t3[:].rearrange("p (c f) -> p c f", c=4),
                                     v2[:, 0:8:2, :], v2[:, 1:8:2, :])
                v3 = t3[:].rearrange("p (c f) -> p c f", c=4)
                nc.vector.tensor_max(t4[:].rearrange("p (c f) -> p c f", c=2),
                                     v3[:, 0:4:2, :], v3[:, 1:4:2, :])
                nc.vector.tensor_max(cmx[:], t4[:, 0:256], t4[:, 256:512])

                # ---- s_conv assembly for this chunk's 8 planes ----
                # f-APs: src (dl:8 step 32)(hh:32 step 1); dst (dl:8 step 70)(hh:32 step 2)
                base = k * CP * HP + 3
                src_av = pcs[:].rearrange("p (d hh) -> p d hh", d=CP)
                dst = s_conv[:].rearrange("p (d h) -> p d h", d=DL)[:, k * CP:(k + 1) * CP, :]
                # avg, even h (aligned rows 0:64): psum -> s_conv rows 0:64
                nc.scalar.activation(
                    dst[0:64, :, 3:67:2], src_av[0:64], Act.Copy, scale=1.0 / 32.0)
                # avg, odd h: psum rows 64:128 -> sbuf (aligned), DMA shift to rows 0:64
                tmp_av = small.tile([128, PFC], bf16, tag="tmpav")
                nc.scalar.activation(tmp_av[64:128, :], pcs[64:128, :], Act.Copy,
                                     scale=1.0 / 32.0)
                sh1 = shiftp.tile([128, PFC], bf16, tag="sh1")
                nc.gpsimd.dma_start(sh1[0:64, :], tmp_av[64:128, :])
                nc.vector.tensor_copy(
                    dst[0:64, :, 4:68:2],
                    sh1[0:64].rearrange("p (d hh) -> p d hh", d=CP))
                # max, odd h (aligned rows 64:128)
                nc.vector.tensor_copy(
                    dst[64:128, :, 4:68:2],
                    cmx[64:128].rearrange("p (d hh) -> p d hh", d=CP))
                # max, even h: shift rows 0:64 -> 64:128
                sh2 = shiftp.tile([128, PFC], bf16, tag="sh2")
                nc.gpsimd.dma_start(sh2[64:128, :], cmx[0:64, :])
                nc.vector.tensor_copy(
                    dst[64:128, :, 3:67:2],
                    sh2[64:128].rearrange("p (d hh) -> p d hh", d=CP))

            def conv_group(g):
                # outputs own planes d_own in [8g, 8g+8) = local d in [8g+4, 8g+12)
                pc_a = pconvp.tile([128, 512], f32, tag="pconv")
                pc_b = pconvp.tile([128, 512], f32, tag="pconv")
                pc = [pc_a, pc_b]
                sc = s_conv[:].rearrange("p (d h) -> p d h", d=DL)
                for t in range(49):
                    kz, ky = t // 7, t % 7
                    d0 = 8 * g + 4 + kz - 3
                    rhs = sc[:, d0:d0 + 8, ky:ky + 64]
                    for pair in range(2):
                        tt = t * 2 + pair
                        nc.tensor.matmul(pc[pair][:],
                                         convw[:, tt * 128:(tt + 1) * 128], rhs,
                                         start=(t == 0), stop=(t == 48),
                                         skip_group_check=True)
                # relu -> sbuf
                for pair in range(2):
                    r = relup.tile([128, 512], bf16, tag="relu")
                    nc.scalar.activation(r[:], pc[pair][:], Act.Relu)
                    relu_tiles[g][pair] = r
                # conv2 (1x1x1, 4 -> 1) and sigmoid
                psa = pmiscp.tile([64, 512], f32, tag="m")
                nc.tensor.matmul(psa[:], c2w[:, 0:64], relu_tiles[g][0][:],
                                 start=True, stop=False, skip_group_check=True)
                nc.tensor.matmul(psa[:], c2w[:, 64:128], relu_tiles[g][1][:],
                                 start=False, stop=True, skip_group_check=True)
                sa_w = sawp.tile([64, 512], f32, tag="saw")
                nc.scalar.activation(sa_w[:], psa[:], Act.Sigmoid)
                # transpose [64,128] blocks -> sa128
                for b4 in range(4):
                    pt = ptpp.tile([128, 64], f32, tag="ptp")
                    nc.tensor.transpose(pt[:], sa_w[:, b4 * 128:(b4 + 1) * 128],
                                        ident[0:64, 0:64])
                    col = (4 * g + b4) * 64
                    nc.scalar.copy(sa128[:, col:col + 64], pt[:])

            def ca_pre():
                # spatial-max: combine chunk partials -> [128, 32] (f32)
                smx = statp.tile([128, 32], f32)
                nc.vector.tensor_reduce(
                    smx[:], spmax_parts[:].rearrange("p (k c) -> p c k", k=NCHUNK),
                    axis=Ax.X, op=Alu.max)
                # partition-axis max on GpSimd (no PE involvement)
                from concourse import bass_isa
                smx_ar = statp.tile([128, 32], f32)
                nc.gpsimd.partition_all_reduce(smx_ar[:], smx[:], 128,
                                               bass_isa.ReduceOp.max)
                spmax_row = smx_ar[0:1, :]
                # per-channel spatial sums: free-axis reduce straight from PSUM
                spsum_col = statp.tile([32, 1], f32)
                nc.vector.tensor_reduce(spsum_col[:], psum_sp[:], axis=Ax.X,
                                        op=Alu.add)
                # assemble the [2, 32] collective payload directly in DRAM
                nc.gpsimd.dma_start(cc_in[0:1, :], spsum_col[:])
                nc.gpsimd.dma_start(cc_in[1:2, :], spmax_row)
                nc.gpsimd.collective_compute(
                    "AllGather", mybir.AluOpType.bypass,
                    replica_groups=[list(range(NCORES))],
                    ins=[cc_in[:].opt()], outs=[cc_out[:].opt()])
                gath = statp.tile([16, 32], f32)
                nc.gpsimd.dma_start(gath[:], cc_out[:])
                return gath

            def ca_post(gath):
                from concourse import bass_isa
                # rank-dependent pair-combine: mask-multiply + partition reduce
                tS = statp.tile([16, 32], f32)
                nc.vector.tensor_scalar_mul(tS[:], gath[:], masks[:, 0:1])
                tSa = statp.tile([16, 32], f32)
                nc.gpsimd.partition_all_reduce(tSa[:], tS[:], 16,
                                               bass_isa.ReduceOp.add)
                tM = statp.tile([16, 32], f32)
                nc.vector.tensor_scalar_mul(tM[:], gath[:], masks[:, 1:2])
                tMa = statp.tile([16, 32], f32)
                nc.gpsimd.partition_all_reduce(tMa[:], tM[:], 16,
                                               bass_isa.ReduceOp.max)
                hin = statp.tile([1, 64], f32)
                nc.vector.tensor_copy(hin[:, 0:32], tSa[0:1, :])
                nc.vector.tensor_copy(hin[:, 32:64], tMa[0:1, :])
                # MLP via broadcast + fused mul-accumulate (no TensorE)
                hinb = statp.tile([128, 64], f32)
                nc.gpsimd.partition_broadcast(hinb[:], hin[:])
                junk1 = statp.tile([128, 64], f32)
                h1 = statp.tile([128, 1], f32)
                nc.vector.scalar_tensor_tensor(junk1[:], fc1w[:], 1.0, hinb[:],
                                               op0=Alu.bypass, op1=Alu.mult,
                                               accum_out=h1[:])
                hrelu = statp.tile([128, 1], f32)
                nc.scalar.activation(hrelu[:], h1[:], Act.Relu, bias=fc1b[:])
                hrow = statp.tile([1, 128], f32)
                nc.gpsimd.dma_start(hrow[:], hrelu[:])
                hb = statp.tile([32, 128], f32)
                nc.gpsimd.partition_broadcast(hb[:], hrow[:])
                junk2 = statp.tile([32, 128], f32)
                ca0 = statp.tile([32, 1], f32)
                nc.vector.scalar_tensor_tensor(junk2[:], fc2w[:], 1.0, hb[:],
                                               op0=Alu.bypass, op1=Alu.mult,
                                               accum_out=ca0[:])
                ca_col = statp.tile([32, 1], f32)
                nc.scalar.activation(ca_col[:], ca0[:], Act.Sigmoid, bias=fc2b[:])
                ca_row = statp.tile([1, 32], f32)
                nc.gpsimd.dma_start(ca_row[:], ca_col[:])
                nc.gpsimd.partition_broadcast(ca_rep[:], ca_row[:])
                nc.scalar.activation(nca_rep[:], ca_rep[:], Act.Copy, scale=-1.0)

            def output_quarter(g, dve_all):
                # outputs for d_own in [8g, 8g+8): sa128 cols [g*256, (g+1)*256)
                sl_sa = slice(g * 256, (g + 1) * 256)
                sa_b4 = sa128[:, sl_sa].rearrange("p (o f) -> p o f", o=1)\
                    .to_broadcast((128, 4, 256))
                for cg in range(8):
                    abuf = outp.tile([128, 1024], f32, tag="abuf")
                    bbuf = outp.tile([128, 1024], f32, tag="bbuf")
                    if cg < 4 and not dve_all:
                        for c4 in range(4):
                            c = cg * 4 + c4
                            sl = slice(c4 * 256, (c4 + 1) * 256)
                            nc.scalar.activation(abuf[:, sl], sa128[:, sl_sa],
                                                 Act.Copy, scale=ca_rep[:, c:c + 1])
                    else:
                        # one tensor_tensor covering 4 channels via broadcast APs
                        ca4 = ca_rep[:, cg * 4:(cg + 1) * 4].to_broadcast(
                            (128, 4, 256))
                        nc.vector.tensor_tensor(
                            abuf[:].rearrange("p (c f) -> p c f", c=4),
                            sa_b4, ca4, op=Alu.mult)
                    nc.vector.tensor_scalar(bbuf[:], abuf[:], -1.0, 1.0,
                                            op0=Alu.mult, op1=Alu.add)
                    nc.scalar.dma_start(attn_ext[g, cg], abuf[:])
                    nc.sync.dma_start(anti_ext[g, cg], bbuf[:])

            # ---- schedule ----
            for k in range(NCHUNK):
                stage1_chunk(k)
            gath = ca_pre()
            conv_group(0)
            conv_group(1)
            ca_post(gath)
            output_quarter(0, dve_all=True)
            output_quarter(1, dve_all=True)
            conv_group(2)
            output_quarter(2, dve_all=False)
            conv_group(3)
            output_quarter(3, dve_all=False)

    nc.compile()
    return nc


def _host_inputs(x, fc1_w, fc1_b, fc2_w, fc2_b, conv1_w, conv2_w):
    """Build the per-core input maps (all host-side numpy)."""
    x = np.asarray(x, dtype=np.float32)
    # conv1 Toeplitz lhsT blocks: T[t2][(i,w_in), (o2,w_out)]
    w1 = np.asarray(conv1_w, dtype=np.float32)  # [4, 2, 7, 7, 7]
    T = np.zeros((98, 128, 128), np.float32)
    for kz in range(7):
        for ky in range(7):
            t = kz * 7 + ky
            for pair in range(2):
                t2 = t * 2 + pair
                for o2 in range(2):
                    oc = pair * 2 + o2
                    for i in range(2):
                        for dk in range(7):
                            off = dk - 3  # w_in = w_out + off
                            wv = w1[oc, i, kz, ky, dk]
                            if off >= 0:
                                wo = np.arange(0, 64 - off)
                            else:
                                wo = np.arange(-off, 64)
                            T[t2, i * 64 + wo + off, o2 * 64 + wo] = wv
    convw = np.ascontiguousarray(T.transpose(1, 0, 2).reshape(128, 98 * 128)).astype(BF16)

    oh = np.zeros((128, 32 * 32), BF16)
    for c in range(32):
        oh[:, c * 32 + c] = 1.0
    ident = np.eye(128, dtype=np.float32)
    identb = np.eye(128, dtype=np.float32).astype(BF16)

    c2v = np.asarray(conv2_w, dtype=np.float32).reshape(4)
    c2 = np.zeros((128, 128), np.float32)
    for pair in range(2):
        for o2 in range(2):
            w = np.arange(64)
            c2[o2 * 64 + w, pair * 64 + w] = c2v[pair * 2 + o2]
    c2 = c2.astype(BF16)

    fc1_w = np.asarray(fc1_w, np.float32)           # [128, 64]
    fc1s = fc1_w.copy()
    fc1s[:, 0:32] *= 1.0 / NVOX
    fc1bv = np.asarray(fc1_b, np.float32).reshape(128, 1)
    fc2v = np.ascontiguousarray(np.asarray(fc2_w, np.float32))     # [32, 128]
    fc2bv = np.asarray(fc2_b, np.float32).reshape(32, 1)

    in_maps = []
    for r in range(NCORES):
        b, dhalf = r // 2, r % 2
        xp = np.zeros((C, DL, H, W), np.float32)
        if dhalf == 0:
            xp[:, 4:40] = x[b, :, 0:36]
        else:
            xp[:, 0:36] = x[b, :, 28:64]
        # [c, k, dl, hh, h2, w] -> [k, h2, w, c, dl, hh] -> [5, 128, 8192]
        xr = xp.reshape(C, NCHUNK, CP, 32, 2, W).transpose(1, 4, 5, 0, 2, 3)
        xhost = np.ascontiguousarray(xr.reshape(NCHUNK, 128, 32 * PFC)).astype(BF16)

        partner = r ^ 1
        masks = np.zeros((16, 2), np.float32)
        masks[2 * r, 0] = 1.0
        masks[2 * partner, 0] = 1.0
        masks[2 * r + 1, 1] = 1.0
        masks[2 * partner + 1, 1] = 1.0

        in_maps.append({
            "x": xhost, "convw": convw, "oh": oh, "ident": ident, "identb": identb, "c2w": c2,
            "fc1w": fc1s, "fc1b": fc1bv, "fc2w": fc2v, "fc2b": fc2bv,
            "masks": masks,
        })
    return in_maps


def _decode_out(arr):
    """[4, 8, 128, 1024] -> [C, 32, H, W] (own planes)."""
    a = arr.reshape(4, 8, 2, 64, 4, 4, 64)          # g, cg, d2, h, c4, dl, w
    a = a.transpose(1, 4, 0, 5, 2, 3, 6)            # cg, c4, g, dl, d2, h, w
    return a.reshape(C, 32, H, W)


def _install_ntff_shim():
    """The agent image's antenv lacks axon_hooks; recreate it so
    run_bass_kernel_spmd(trace=True) can NTFF-profile via libaxon."""
    import sys, types, contextlib, ctypes
    try:
        import antenv.axon_hooks  # noqa
        return
    except ImportError:
        pass
    so_path = "/opt/axon/libaxon_pjrt.so"
    lib = ctypes.CDLL(so_path)
    if not hasattr(lib, "axon_start_nrt_profile"):
        return
    lib.axon_start_nrt_profile.argtypes = [ctypes.POINTER(ctypes.c_int64),
                                           ctypes.c_size_t]
    lib.axon_start_nrt_profile.restype = ctypes.c_int64
    lib.axon_stop_nrt_profile.argtypes = [ctypes.c_char_p]
    lib.axon_stop_nrt_profile.restype = ctypes.c_int64

    @contextlib.contextmanager
    def _hook(output_dir, device_ids):
        import jax
        jax.devices()
        if device_ids:
            ids = (ctypes.c_int64 * len(device_ids))(*device_ids)
            rc = lib.axon_start_nrt_profile(ids, len(device_ids))
        else:
            rc = lib.axon_start_nrt_profile(None, 0)
        if rc != 0:
            raise RuntimeError(f"axon_start_nrt_profile rc={rc}")
        try:
            yield
        finally:
            n = lib.axon_stop_nrt_profile(str(output_dir).encode())
            print(f"profile: {n} file(s) written to {output_dir}")

    mod = types.ModuleType("antenv.axon_hooks")
    _state = {"hook": _hook}
    mod.get_axon_ntff_profile_hook = lambda: _state["hook"]
    mod.set_axon_ntff_profile_hook = lambda h: _state.__setitem__("hook", h)
    sys.modules["antenv.axon_hooks"] = mod


def kernel(x, fc1_w, fc1_b, fc2_w, fc2_b, conv1_w, conv2_w, _want_time=False):
    from concourse.bass_utils import run_bass_kernel_spmd
    if _want_time:
        _install_ntff_shim()

    if "nc" not in _CACHE:
        _CACHE["nc"] = _build_nc()
    nc = _CACHE["nc"]

    in_maps = _host_inputs(x, fc1_w, fc1_b, fc2_w, fc2_b, conv1_w, conv2_w)
    res = run_bass_kernel_spmd(nc, in_maps, core_ids=list(range(NCORES)),
                               trace=bool(_want_time))
    attention = np.empty((B, C, D, H, W), np.float32)
    anti = np.empty((B, C, D, H, W), np.float32)
    for r in range(NCORES):
        b, dhalf = r // 2, r % 2
        d0 = dhalf * 32
        attention[b, :, d0:d0 + 32] = _decode_out(res.results[r]["attn"])
        anti[b, :, d0:d0 + 32] = _decode_out(res.results[r]["anti"])
    if _want_time:
        return (attention, anti), res.exec_time_ns
    return attention, anti


# revision 16
# speedup vs baseline: 1.0085x; 1.0085x over previous
"""Trainium2 Bass kernel for nn_Attention (CBAM-style channel+spatial attention).

Computes, for x [4, 32, 64, 64, 64]:
  ca[b, c]       = sigmoid(MLP(concat(mean_dhw(x), max_dhw(x))))
  sa[b, d, h, w] = sigmoid(conv2(relu(conv1(concat(mean_c(x), max_c(x))))))
  attention      = sa * ca;  anti_attention = 1 - attention

Sharded over 8 NeuronCores as (batch, D-half); each core gets a host-padded
40-plane slab (4 halo planes each side) pre-rearranged into the on-chip
layout.  Cross-core traffic is one AllGather of 64 stats floats.
"""
import numpy as np
import ml_dtypes

BF16 = ml_dtypes.bfloat16

B, C, D, H, W = 4, 32, 64, 64, 64
K = 7
NCORES = 8
HALO = 4
DL = 40            # local planes per core (32 own + 2*4 halo)
NCHUNK = 5         # 8-plane chunks
CP = 8             # planes per chunk
PFC = CP * 32      # f-cols per channel per chunk (d_loc*32 + h//2) = 256
HP = H + 6         # padded h extent in s_conv (70)
NVOX = float(D * H * W)

_CACHE = {}


def _build_nc():
    import concourse.bacc as bacc
    import concourse.mybir as mybir
    from concourse import tile

    f32 = mybir.dt.float32
    bf16 = mybir.dt.bfloat16
    Alu = mybir.AluOpType
    Act = mybir.ActivationFunctionType
    Ax = mybir.AxisListType

    nc = bacc.Bacc("TRN2", target_bir_lowering=False, debug=False,
                   num_devices=NCORES)

    # ---- external I/O ----
    x_ext = nc.declare_dram_parameter("x", [NCHUNK, 128, 32 * PFC], bf16, isOutput=False)
    convw_ext = nc.declare_dram_parameter("convw", [128, 98 * 128], bf16, isOutput=False)
    oh_ext = nc.declare_dram_parameter("oh", [128, 32 * 32], bf16, isOutput=False)
    id_ext = nc.declare_dram_parameter("ident", [128, 128], f32, isOutput=False)
    idb_ext = nc.declare_dram_parameter("identb", [128, 128], bf16, isOutput=False)
    c2_ext = nc.declare_dram_parameter("c2w", [128, 128], bf16, isOutput=False)
    fc1w_ext = nc.declare_dram_parameter("fc1w", [128, 64], f32, isOutput=False)
    fc1b_ext = nc.declare_dram_parameter("fc1b", [128, 1], f32, isOutput=False)
    fc2w_ext = nc.declare_dram_parameter("fc2w", [32, 128], f32, isOutput=False)
    fc2b_ext = nc.declare_dram_parameter("fc2b", [32, 1], f32, isOutput=False)
    mask_ext = nc.declare_dram_parameter("masks", [16, 2], f32, isOutput=False)
    attn_ext = nc.declare_dram_parameter("attn", [4, 8, 128, 1024], f32, isOutput=True)
    anti_ext = nc.declare_dram_parameter("anti", [4, 8, 128, 1024], f32, isOutput=True)

    cc_in = nc.dram_tensor("cc_in", [2, 32], f32)
    cc_out = nc.dram_tensor("cc_out", [16, 32], f32, addr_space="Shared")

    with tile.TileContext(nc) as tc:
        with (
            tc.tile_pool(name="consts", bufs=1) as consts,
            tc.tile_pool(name="xpool", bufs=2) as xpool,
            tc.tile_pool(name="sconv", bufs=1) as sconvp,
            tc.tile_pool(name="small", bufs=2) as small,
            tc.tile_pool(name="tree", bufs=1) as treep,
            tc.tile_pool(name="shift", bufs=2) as shiftp,
            tc.tile_pool(name="relu", bufs=2) as relup,
            tc.tile_pool(name="saw", bufs=2) as sawp,
            tc.tile_pool(name="stat", bufs=1) as statp,
            tc.tile_pool(name="outp", bufs=2) as outp,
            tc.tile_pool(name="pcs", bufs=1, space="PSUM") as pcsp,
            tc.tile_pool(name="psp", bufs=1, space="PSUM") as pspp,
            tc.tile_pool(name="pconv", bufs=4, space="PSUM") as pconvp,
            tc.tile_pool(name="ptp", bufs=1, space="PSUM") as ptpp,
            tc.tile_pool(name="pmisc", bufs=1, space="PSUM") as pmiscp,
        ):
            # ---- constants ----
            oh = consts.tile([128, 32 * 32], bf16)
            nc.gpsimd.dma_start(oh[:], oh_ext[:])
            ident = consts.tile([128, 128], f32)
            nc.gpsimd.dma_start(ident[:], id_ext[:])
            identb = consts.tile([128, 128], bf16)
            nc.gpsimd.dma_start(identb[:], idb_ext[:])
            c2w = consts.tile([128, 128], bf16)
            nc.gpsimd.dma_start(c2w[:], c2_ext[:])
            fc1w = consts.tile([128, 64], f32)
            nc.gpsimd.dma_start(fc1w[:], fc1w_ext[:])
            fc1b = consts.tile([128, 1], f32)
            nc.gpsimd.dma_start(fc1b[:], fc1b_ext[:])
            fc2w = consts.tile([32, 128], f32)
            nc.gpsimd.dma_start(fc2w[:], fc2w_ext[:])
            fc2b = consts.tile([32, 1], f32)
            nc.gpsimd.dma_start(fc2b[:], fc2b_ext[:])
            masks = consts.tile([16, 2], f32)
            nc.gpsimd.dma_start(masks[:], mask_ext[:])
            convw = consts.tile([128, 98 * 128], bf16)
            nc.gpsimd.dma_start(convw[:], convw_ext[:])
            ones1 = consts.tile([1, 128], f32)
            nc.vector.memset(ones1[:], 1.0)

            # warm the ACT sigmoid/relu table set off the critical path
            warm = consts.tile([1, 1], f32)
            nc.vector.memset(warm[:], 0.0)
            warm2 = consts.tile([1, 1], f32)
            nc.scalar.activation(warm2[:], warm[:], Act.Sigmoid)

            # persistent accumulators / results
            s_conv = sconvp.tile([128, DL * HP], bf16)       # rows: i*64+w; f: d*70+3+h
            nc.vector.memset(s_conv[:], 0.0)
            spmax_parts = statp.tile([128, 32 * NCHUNK], f32)
            sa128 = statp.tile([128, 1024], f32)            # p=(do%2)*64+h, f=(do//2)*64+w
            ca_rep = statp.tile([128, 32], f32)
            nca_rep = statp.tile([128, 32], f32)
            psum_sp = pspp.tile([32, 256], f32)             # per-channel spatial sums
            spsum_col = statp.tile([32, 1], f32)

            relu_tiles = [[None, None] for _ in range(4)]
            sp_first = [True]

            def stage1_chunk(k):
                x_k = xpool.tile([128, 32 * PFC], bf16, tag="xk")
                if k == 0:
                    for q in range(4):
                        eng = nc.sync if q % 2 == 0 else nc.scalar
                        eng.dma_start(x_k[:, q * 2048:(q + 1) * 2048],
                                      x_ext[0, :, q * 2048:(q + 1) * 2048])
                else:
                    eng = nc.sync if k % 2 == 0 else nc.scalar
                    eng.dma_start(x_k[:], x_ext[k])

                # channel-sum (identity-matmul accumulation over the 32 channels)
                pcs = pcsp.tile([128, PFC], f32, tag="pcs")
                for c in range(32):
                    nc.tensor.matmul(pcs[:], identb[:], x_k[:, c * PFC:(c + 1) * PFC],
                                     start=(c == 0), stop=(c == 31),
                                     skip_group_check=True)

                # per-channel spatial sums over own planes -> psum_sp (accumulates)
                off, end = (128, 256) if k == 0 else ((0, 128) if k == 4 else (0, 256))
                n = end - off
                for c in range(32):
                    nc.tensor.matmul(psum_sp[:, off:end], oh[:, c * 32:(c + 1) * 32],
                                     x_k[:, c * PFC + off: c * PFC + end],
                                     start=sp_first[0],
                                     stop=(k == 4 and c == 31),
                                     skip_group_check=True)
                    sp_first[0] = False

                if k == 4:
                    # per-channel spatial sums are complete once this chunk's
                    # sp matmuls land; reduce + ship to DRAM as early as we can
                    nc.vector.tensor_reduce(spsum_col[:], psum_sp[:], axis=Ax.X,
                                            op=Alu.add)
                    nc.gpsimd.dma_start(cc_in[0:1, :], spsum_col[:])
                # per-channel spatial max over own planes: one strided reduce
                # (emitted first: the ca stats gate the whole output phase)
                nc.vector.tensor_reduce(
                    spmax_parts[:, k * 32:(k + 1) * 32],
                    x_k[:].rearrange("p (c f) -> p c f", c=32)[:, :, off:end],
                    axis=Ax.X, op=Alu.max)

                # channel-max: binary tensor_max tree (bf16 runs at 2x mode)
                t1 = treep.tile([128, 4096], bf16, tag="tr1")
                t2 = treep.tile([128, 2048], bf16, tag="tr2")
                t3 = treep.tile([128, 1024], bf16, tag="tr3")
                t4 = treep.tile([128, 512], bf16, tag="tr4")
                cmx = small.tile([128, PFC], bf16, tag="cmx")
                xv = x_k[:].rearrange("p (c f) -> p c f", c=32)
                nc.vector.tensor_max(t1[:].rearrange("p (c f) -> p c f", c=16),
                                     xv[:, 0:32:2, :], xv[:, 1:32:2, :])
                v1 = t1[:].rearrange("p (c f) -> p c f", c=16)
                nc.vector.tensor_max(t2[:].rearrange("p (c f) -> p c f", c=8),
                                     v1[:, 0:16:2, :], v1[:, 1:16:2, :])
                v2 = t2[:].rearrange("p (c f) -> p c f", c=8)
                nc.vector.tensor_max(t3[:].rearrange("p (c f) -> p c f", c=4),
                                     v2[:, 0:8:2, :], v2[:, 1:8:2, :])
                v3 = t3[:].rearrange("p (c f) -> p c f", c=4)
                nc.vector.tensor_max(t4[:].rearrange("p (c f) -> p c f", c=2),
                                     v3[:, 0:4:2, :], v3[:, 1:4:2, :])
                nc.vector.tensor_max(cmx[:], t4[:, 0:256], t4[:, 256:512])

                # ---- s_conv assembly for this chunk's 8 planes ----
                # f-APs: src (dl:8 step 32)(hh:32 step 1); dst (dl:8 step 70)(hh:32 step 2)
                base = k * CP * HP + 3
                src_av = pcs[:].rearrange("p (d hh) -> p d hh", d=CP)
                dst = s_conv[:].rearrange("p (d h) -> p d h", d=DL)[:, k * CP:(k + 1) * CP, :]
                # avg, even h (aligned rows 0:64): psum -> s_conv rows 0:64
                nc.scalar.activation(
                    dst[0:64, :, 3:67:2], src_av[0:64], Act.Copy, scale=1.0 / 32.0)
                # avg, odd h: psum rows 64:128 -> sbuf (aligned), DMA shift to rows 0:64
                tmp_av = small.tile([128, PFC], bf16, tag="tmpav")
                nc.scalar.activation(tmp_av[64:128, :], pcs[64:128, :], Act.Copy,
                                     scale=1.0 / 32.0)
                sh1 = shiftp.tile([128, PFC], bf16, tag="sh1")
                nc.gpsimd.dma_start(sh1[0:64, :], tmp_av[64:128, :])
                nc.vector.tensor_copy(
                    dst[0:64, :, 4:68:2],
                    sh1[0:64].rearrange("p (d hh) -> p d hh", d=CP))
                # max, odd h (aligned rows 64:128)
                nc.vector.tensor_copy(
                    dst[64:128, :, 4:68:2],
                    cmx[64:128].rearrange("p (d hh) -> p d hh", d=CP))
                # max, even h: shift rows 0:64 -> 64:128
                sh2 = shiftp.tile([128, PFC], bf16, tag="sh2")
                nc.gpsimd.dma_start(sh2[64:128, :], cmx[0:64, :])
                nc.vector.tensor_copy(
                    dst[64:128, :, 3:67:2],
                    sh2[64:128].rearrange("p (d hh) -> p d hh", d=CP))

            def conv_group(g):
                # outputs own planes d_own in [8g, 8g+8) = local d in [8g+4, 8g+12)
                pc_a = pconvp.tile([128, 512], f32, tag="pconv")
                pc_b = pconvp.tile([128, 512], f32, tag="pconv")
                pc = [pc_a, pc_b]
                sc = s_conv[:].rearrange("p (d h) -> p d h", d=DL)
                for t in range(49):
                    kz, ky = t // 7, t % 7
                    d0 = 8 * g + 4 + kz - 3
                    rhs = sc[:, d0:d0 + 8, ky:ky + 64]
                    for pair in range(2):
                        tt = t * 2 + pair
                        nc.tensor.matmul(pc[pair][:],
                                         convw[:, tt * 128:(tt + 1) * 128], rhs,
                                         start=(t == 0), stop=(t == 48),
                                         skip_group_check=True)
                # relu -> sbuf
                for pair in range(2):
                    r = relup.tile([128, 512], bf16, tag="relu")
                    nc.scalar.activation(r[:], pc[pair][:], Act.Relu)
                    relu_tiles[g][pair] = r
                # conv2 (1x1x1, 4 -> 1) and sigmoid
                psa = pmiscp.tile([64, 512], f32, tag="m")
                nc.tensor.matmul(psa[:], c2w[:, 0:64], relu_tiles[g][0][:],
                                 start=True, stop=False, skip_group_check=True)
                nc.tensor.matmul(psa[:], c2w[:, 64:128], relu_tiles[g][1][:],
                                 start=False, stop=True, skip_group_check=True)
                sa_w = sawp.tile([64, 512], f32, tag="saw")
                nc.scalar.activation(sa_w[:], psa[:], Act.Sigmoid)
                # transpose [64,128] blocks -> sa128
                for b4 in range(4):
                    pt = ptpp.tile([128, 64], f32, tag="ptp")
                    nc.tensor.transpose(pt[:], sa_w[:, b4 * 128:(b4 + 1) * 128],
                                        ident[0:64, 0:64])
                    col = (4 * g + b4) * 64
                    nc.scalar.copy(sa128[:, col:col + 64], pt[:])

            def ca_pre():
                # spatial-max: combine chunk partials -> [128, 32] (f32)
                smx = statp.tile([128, 32], f32)
                nc.vector.tensor_reduce(
                    smx[:], spmax_parts[:].rearrange("p (k c) -> p c k", k=NCHUNK),
                    axis=Ax.X, op=Alu.max)
                # partition-axis max on GpSimd (no PE involvement)
                from concourse import bass_isa
                smx_ar = statp.tile([128, 32], f32)
                nc.gpsimd.partition_all_reduce(smx_ar[:], smx[:], 128,
                                               bass_isa.ReduceOp.max)
                spmax_row = smx_ar[0:1, :]
                nc.gpsimd.dma_start(cc_in[1:2, :], spmax_row)
                nc.gpsimd.collective_compute(
                    "AllGather", mybir.AluOpType.bypass,
                    replica_groups=[list(range(NCORES))],
                    ins=[cc_in[:].opt()], outs=[cc_out[:].opt()])
                gath = statp.tile([16, 32], f32)
                nc.gpsimd.dma_start(gath[:], cc_out[:])
                return gath

            def ca_post(gath):
                from concourse import bass_isa
                # rank-dependent pair-combine: mask-multiply + partition reduce
                tS = statp.tile([16, 32], f32)
                nc.vector.tensor_scalar_mul(tS[:], gath[:], masks[:, 0:1])
                tSa = statp.tile([16, 32], f32)
                nc.gpsimd.partition_all_reduce(tSa[:], tS[:], 16,
                                               bass_isa.ReduceOp.add)
                tM = statp.tile([16, 32], f32)
                nc.vector.tensor_scalar_mul(tM[:], gath[:], masks[:, 1:2])
                tMa = statp.tile([16, 32], f32)
                nc.gpsimd.partition_all_reduce(tMa[:], tM[:], 16,
                                               bass_isa.ReduceOp.max)
                hin = statp.tile([1, 64], f32)
                nc.vector.tensor_copy(hin[:, 0:32], tSa[0:1, :])
                nc.vector.tensor_copy(hin[:, 32:64], tMa[0:1, :])
                # MLP via broadcast + fused mul-accumulate (no TensorE)
                hinb = statp.tile([128, 64], f32)
                nc.gpsimd.partition_broadcast(hinb[:], hin[:])
                junk1 = statp.tile([128, 64], f32)
                h1 = statp.tile([128, 1], f32)
                nc.vector.scalar_tensor_tensor(junk1[:], fc1w[:], 1.0, hinb[:],
                                               op0=Alu.bypass, op1=Alu.mult,
                                               accum_out=h1[:])
                hrelu = statp.tile([128, 1], f32)
                nc.vector.tensor_scalar(hrelu[:], h1[:], fc1b[:], 0.0,
                                        op0=Alu.add, op1=Alu.max)
                hrow = statp.tile([1, 128], f32)
                nc.gpsimd.dma_start(hrow[:], hrelu[:])
                hb = statp.tile([32, 128], f32)
                nc.gpsimd.partition_broadcast(hb[:], hrow[:])
                junk2 = statp.tile([32, 128], f32)
                ca0 = statp.tile([32, 1], f32)
                nc.vector.scalar_tensor_tensor(junk2[:], fc2w[:], 1.0, hb[:],
                                               op0=Alu.bypass, op1=Alu.mult,
                                               accum_out=ca0[:])
                ca_col = statp.tile([32, 1], f32)
                nc.scalar.activation(ca_col[:], ca0[:], Act.Sigmoid, bias=fc2b[:])
                ca_row = statp.tile([1, 32], f32)
                nc.gpsimd.dma_start(ca_row[:], ca_col[:])
                nc.gpsimd.partition_broadcast(ca_rep[:], ca_row[:])
                nc.vector.tensor_scalar_mul(nca_rep[:], ca_rep[:], -1.0)

            def output_quarter(g, dve_all):
                # outputs for d_own in [8g, 8g+8): sa128 cols [g*256, (g+1)*256)
                sl_sa = slice(g * 256, (g + 1) * 256)
                sa_b4 = sa128[:, sl_sa].rearrange("p (o f) -> p o f", o=1)\
                    .to_broadcast((128, 4, 256))
                for cg in range(8):
                    abuf = outp.tile([128, 1024], f32, tag="abuf")
                    bbuf = outp.tile([128, 1024], f32, tag="bbuf")
                    if cg < 4 and not dve_all:
                        for c4 in range(4):
                            c = cg * 4 + c4
                            sl = slice(c4 * 256, (c4 + 1) * 256)
                            nc.scalar.activation(abuf[:, sl], sa128[:, sl_sa],
                                                 Act.Copy, scale=ca_rep[:, c:c + 1])
                    else:
                        # one tensor_tensor covering 4 channels via broadcast APs
                        ca4 = ca_rep[:, cg * 4:(cg + 1) * 4].to_broadcast(
                            (128, 4, 256))
                        nc.vector.tensor_tensor(
                            abuf[:].rearrange("p (c f) -> p c f", c=4),
                            sa_b4, ca4, op=Alu.mult)
                    nc.vector.tensor_scalar(bbuf[:], abuf[:], -1.0, 1.0,
                                            op0=Alu.mult, op1=Alu.add)
                    nc.scalar.dma_start(attn_ext[g, cg], abuf[:])
                    nc.sync.dma_start(anti_ext[g, cg], bbuf[:])

            # ---- schedule ----
            for k in range(NCHUNK):
                stage1_chunk(k)
            gath = ca_pre()
            conv_group(0)
            conv_group(1)
            conv_group(2)
            ca_post(gath)
            output_quarter(0, dve_all=True)
            output_quarter(1, dve_all=True)
            output_quarter(2, dve_all=False)
            conv_group(3)
            output_quarter(3, dve_all=False)

    nc.compile()
    return nc


def _host_inputs(x, fc1_w, fc1_b, fc2_w, fc2_b, conv1_w, conv2_w):
    """Build the per-core input maps (all host-side numpy)."""
    x = np.asarray(x, dtype=np.float32)
    # conv1 Toeplitz lhsT blocks: T[t2][(i,w_in), (o2,w_out)]
    w1 = np.asarray(conv1_w, dtype=np.float32)  # [4, 2, 7, 7, 7]
    T = np.zeros((98, 128, 128), np.float32)
    for kz in range(7):
        for ky in range(7):
            t = kz * 7 + ky
            for pair in range(2):
                t2 = t * 2 + pair
                for o2 in range(2):
                    oc = pair * 2 + o2
                    for i in range(2):
                        for dk in range(7):
                            off = dk - 3  # w_in = w_out + off
                            wv = w1[oc, i, kz, ky, dk]
                            if off >= 0:
                                wo = np.arange(0, 64 - off)
                            else:
                                wo = np.arange(-off, 64)
                            T[t2, i * 64 + wo + off, o2 * 64 + wo] = wv
    convw = np.ascontiguousarray(T.transpose(1, 0, 2).reshape(128, 98 * 128)).astype(BF16)

    oh = np.zeros((128, 32 * 32), BF16)
    for c in range(32):
        oh[:, c * 32 + c] = 1.0
    ident = np.eye(128, dtype=np.float32)
    identb = np.eye(128, dtype=np.float32).astype(BF16)

    c2v = np.asarray(conv2_w, dtype=np.float32).reshape(4)
    c2 = np.zeros((128, 128), np.float32)
    for pair in range(2):
        for o2 in range(2):
            w = np.arange(64)
            c2[o2 * 64 + w, pair * 64 + w] = c2v[pair * 2 + o2]
    c2 = c2.astype(BF16)

    fc1_w = np.asarray(fc1_w, np.float32)           # [128, 64]
    fc1s = fc1_w.copy()
    fc1s[:, 0:32] *= 1.0 / NVOX
    fc1bv = np.asarray(fc1_b, np.float32).reshape(128, 1)
    fc2v = np.ascontiguousarray(np.asarray(fc2_w, np.float32))     # [32, 128]
    fc2bv = np.asarray(fc2_b, np.float32).reshape(32, 1)

    in_maps = []
    for r in range(NCORES):
        b, dhalf = r // 2, r % 2
        xp = np.zeros((C, DL, H, W), np.float32)
        if dhalf == 0:
            xp[:, 4:40] = x[b, :, 0:36]
        else:
            xp[:, 0:36] = x[b, :, 28:64]
        # [c, k, dl, hh, h2, w] -> [k, h2, w, c, dl, hh] -> [5, 128, 8192]
        xr = xp.reshape(C, NCHUNK, CP, 32, 2, W).transpose(1, 4, 5, 0, 2, 3)
        xhost = np.ascontiguousarray(xr.reshape(NCHUNK, 128, 32 * PFC)).astype(BF16)

        partner = r ^ 1
        masks = np.zeros((16, 2), np.float32)
        masks[2 * r, 0] = 1.0
        masks[2 * partner, 0] = 1.0
        masks[2 * r + 1, 1] = 1.0
        masks[2 * partner + 1, 1] = 1.0

        in_maps.append({
            "x": xhost, "convw": convw, "oh": oh, "ident": ident, "identb": identb, "c2w": c2,
            "fc1w": fc1s, "fc1b": fc1bv, "fc2w": fc2v, "fc2b": fc2bv,
            "masks": masks,
        })
    return in_maps


def _decode_out(arr):
    """[4, 8, 128, 1024] -> [C, 32, H, W] (own planes)."""
    a = arr.reshape(4, 8, 2, 64, 4, 4, 64)          # g, cg, d2, h, c4, dl, w
    a = a.transpose(1, 4, 0, 5, 2, 3, 6)            # cg, c4, g, dl, d2, h, w
    return a.reshape(C, 32, H, W)


def _install_ntff_shim():
    """The agent image's antenv lacks axon_hooks; recreate it so
    run_bass_kernel_spmd(trace=True) can NTFF-profile via libaxon."""
    import sys, types, contextlib, ctypes
    try:
        import antenv.axon_hooks  # noqa
        return
    except ImportError:
        pass
    so_path = "/opt/axon/libaxon_pjrt.so"
    lib = ctypes.CDLL(so_path)
    if not hasattr(lib, "axon_start_nrt_profile"):
        return
    lib.axon_start_nrt_profile.argtypes = [ctypes.POINTER(ctypes.c_int64),
                                           ctypes.c_size_t]
    lib.axon_start_nrt_profile.restype = ctypes.c_int64
    lib.axon_stop_nrt_profile.argtypes = [ctypes.c_char_p]
    lib.axon_stop_nrt_profile.restype = ctypes.c_int64

    @contextlib.contextmanager
    def _hook(output_dir, device_ids):
        import jax
        jax.devices()
        if device_ids:
            ids = (ctypes.c_int64 * len(device_ids))(*device_ids)
            rc = lib.axon_start_nrt_profile(ids, len(device_ids))
        else:
            rc = lib.axon_start_nrt_profile(None, 0)
        if rc != 0:
            raise RuntimeError(f"axon_start_nrt_profile rc={rc}")
        try:
            yield
        finally:
            n = lib.axon_stop_nrt_profile(str(output_dir).encode())
            print(f"profile: {n} file(s) written to {output_dir}")

    mod = types.ModuleType("antenv.axon_hooks")
    _state = {"hook": _hook}
    mod.get_axon_ntff_profile_hook = lambda: _state["hook"]
    mod.set_axon_ntff_profile_hook = lambda h: _state.__setitem__("hook", h)
    sys.modules["antenv.axon_hooks"] = mod


def kernel(x, fc1_w, fc1_b, fc2_w, fc2_b, conv1_w, conv2_w, _want_time=False):
    from concourse.bass_utils import run_bass_kernel_spmd
    if _want_time:
        _install_ntff_shim()

    if "nc" not in _CACHE:
        _CACHE["nc"] = _build_nc()
    nc = _CACHE["nc"]

    in_maps = _host_inputs(x, fc1_w, fc1_b, fc2_w, fc2_b, conv1_w, conv2_w)
    res = run_bass_kernel_spmd(nc, in_maps, core_ids=list(range(NCORES)),
                               trace=bool(_want_time))
    attention = np.empty((B, C, D, H, W), np.float32)
    anti = np.empty((B, C, D, H, W), np.float32)
    for r in range(NCORES):
        b, dhalf = r // 2, r % 2
        d0 = dhalf * 32
        attention[b, :, d0:d0 + 32] = _decode_out(res.results[r]["attn"])
        anti[b, :, d0:d0 + 32] = _decode_out(res.results[r]["anti"])
    if _want_time:
        return (attention, anti), res.exec_time_ns
    return attention, anti


# revision 30
# speedup vs baseline: 1.3778x; 1.3662x over previous
"""Trainium2 Bass kernel for nn_Attention (CBAM-style channel+spatial attention).

Computes, for x [4, 32, 64, 64, 64]:
  ca[b, c]       = sigmoid(MLP(concat(mean_dhw(x), max_dhw(x))))
  sa[b, d, h, w] = sigmoid(conv2(relu(conv1(concat(mean_c(x), max_c(x))))))
  attention      = sa * ca;  anti_attention = 1 - attention

Sharded over 8 NeuronCores as (batch, D-half); each core gets a host-padded
40-plane slab (4 halo planes each side) pre-rearranged into the on-chip
layout.  Cross-core traffic is one AllGather of 64 stats floats.
"""
import numpy as np
import ml_dtypes

BF16 = np.float16

B, C, D, H, W = 4, 32, 64, 64, 64
K = 7
NCORES = 8
HALO = 4
DL = 40            # local planes per core (32 own + 2*4 halo)
NCHUNK = 5         # 8-plane chunks
CP = 8             # planes per chunk
PFC = CP * 32      # f-cols per channel per chunk (d_loc*32 + h//2) = 256
HP = H + 6         # padded h extent in s_conv (70)
NVOX = float(D * H * W)

_CACHE = {}


def _build_nc():
    import concourse.bacc as bacc
    import concourse.mybir as mybir
    from concourse import tile

    f32 = mybir.dt.float32
    bf16 = mybir.dt.float16
    Alu = mybir.AluOpType
    Act = mybir.ActivationFunctionType
    Ax = mybir.AxisListType

    nc = bacc.Bacc("TRN2", target_bir_lowering=False, debug=False,
                   num_devices=NCORES)

    # ---- external I/O ----
    x_ext = nc.declare_dram_parameter("x", [NCHUNK, 128, 32 * PFC], bf16, isOutput=False)
    convw_ext = nc.declare_dram_parameter("convw", [128, 98 * 128], bf16, isOutput=False)
    oh_ext = nc.declare_dram_parameter("oh", [128, 32 * 32], bf16, isOutput=False)
    id_ext = nc.declare_dram_parameter("ident", [128, 128], f32, isOutput=False)
    idb_ext = nc.declare_dram_parameter("identb", [128, 128], bf16, isOutput=False)
    c2_ext = nc.declare_dram_parameter("c2w", [128, 128], bf16, isOutput=False)
    fc1w_ext = nc.declare_dram_parameter("fc1w", [128, 64], f32, isOutput=False)
    fc1b_ext = nc.declare_dram_parameter("fc1b", [128, 1], f32, isOutput=False)
    fc2w_ext = nc.declare_dram_parameter("fc2w", [32, 128], f32, isOutput=False)
    fc2b_ext = nc.declare_dram_parameter("fc2b", [32, 1], f32, isOutput=False)
    mask_ext = nc.declare_dram_parameter("masks", [16, 2], f32, isOutput=False)
    attn_ext = nc.declare_dram_parameter("attn", [4, 8, 128, 1024], bf16, isOutput=True)
    anti_ext = nc.declare_dram_parameter("anti", [4, 8, 128, 1024], bf16, isOutput=True)

    ccw_in = nc.dram_tensor("ccw_in", [1, 4], f32)
    ccw_out = nc.dram_tensor("ccw_out", [8, 4], f32, addr_space="Shared")
    cc_in = nc.dram_tensor("cc_in", [2, 32], f32)
    cc_out = nc.dram_tensor("cc_out", [16, 32], f32, addr_space="Shared")

    with tile.TileContext(nc) as tc:
        with (
            tc.tile_pool(name="consts", bufs=1) as consts,
            tc.tile_pool(name="xpool", bufs=2) as xpool,
            tc.tile_pool(name="sconv", bufs=1) as sconvp,
            tc.tile_pool(name="small", bufs=2) as small,
            tc.tile_pool(name="tree", bufs=1) as treep,
            tc.tile_pool(name="shift", bufs=2) as shiftp,
            tc.tile_pool(name="relu", bufs=2) as relup,
            tc.tile_pool(name="saw", bufs=2) as sawp,
            tc.tile_pool(name="stat", bufs=1) as statp,
            tc.tile_pool(name="outp", bufs=3) as outp,
            tc.tile_pool(name="pcs", bufs=1, space="PSUM") as pcsp,
            tc.tile_pool(name="psp", bufs=1, space="PSUM") as pspp,
            tc.tile_pool(name="pconv", bufs=4, space="PSUM") as pconvp,
            tc.tile_pool(name="ptp", bufs=1, space="PSUM") as ptpp,
            tc.tile_pool(name="pmisc", bufs=1, space="PSUM") as pmiscp,
        ):
            # ---- constants ----
            oh = consts.tile([128, 32 * 32], bf16)
            nc.gpsimd.dma_start(oh[:], oh_ext[:])
            ident = consts.tile([128, 128], f32)
            nc.gpsimd.dma_start(ident[:], id_ext[:])
            identb = consts.tile([128, 128], bf16)
            nc.gpsimd.dma_start(identb[:], idb_ext[:])
            c2w = consts.tile([128, 128], bf16)
            nc.gpsimd.dma_start(c2w[:], c2_ext[:])
            fc1w = consts.tile([128, 64], f32)
            nc.gpsimd.dma_start(fc1w[:], fc1w_ext[:])
            fc1b = consts.tile([128, 1], f32)
            nc.gpsimd.dma_start(fc1b[:], fc1b_ext[:])
            fc2w = consts.tile([32, 128], f32)
            nc.gpsimd.dma_start(fc2w[:], fc2w_ext[:])
            fc2b = consts.tile([32, 1], f32)
            nc.gpsimd.dma_start(fc2b[:], fc2b_ext[:])
            masks = consts.tile([16, 2], f32)
            nc.gpsimd.dma_start(masks[:], mask_ext[:])
            convw = consts.tile([128, 98 * 128], bf16)
            nc.gpsimd.dma_start(convw[:], convw_ext[:])
            ones1 = consts.tile([1, 128], f32)
            nc.vector.memset(ones1[:], 1.0)

            # warm the ACT sigmoid/relu table set off the critical path
            warm = consts.tile([1, 1], f32)
            nc.vector.memset(warm[:], 0.0)
            warm2 = consts.tile([1, 1], f32)
            nc.scalar.activation(warm2[:], warm[:], Act.Sigmoid)

            # warm the collective path so the real AllGather runs at speed
            warm4 = consts.tile([1, 4], f32)
            nc.vector.memset(warm4[:], 0.0)
            nc.gpsimd.dma_start(ccw_in[:], warm4[:])
            nc.gpsimd.collective_compute(
                "AllGather", mybir.AluOpType.bypass,
                replica_groups=[list(range(NCORES))],
                ins=[ccw_in[:].opt()], outs=[ccw_out[:].opt()])

            # persistent accumulators / results
            s_conv = sconvp.tile([128, DL * HP], bf16)       # rows: i*64+w; f: d*70+3+h
            nc.vector.memset(s_conv[:], 0.0)
            spmax_parts = statp.tile([128, 32 * NCHUNK], f32)
            sa128 = statp.tile([128, 1024], f32)            # p=(do%2)*64+h, f=(do//2)*64+w
            ca_rep = statp.tile([128, 32], f32)
            psum_sp = pspp.tile([32, 256], f32)             # per-channel spatial sums
            spsum_col = statp.tile([32, 1], f32)

            relu_tiles = [[None, None] for _ in range(4)]
            sp_first = [True]

            def stage1_chunk(k):
                x_k = xpool.tile([128, 32 * PFC], bf16, tag="xk")
                nparts = 4 if k == 0 else 2
                step = 8192 // nparts
                for q in range(nparts):
                    eng = nc.sync if q % 2 == 0 else nc.scalar
                    eng.dma_start(x_k[:, q * step:(q + 1) * step],
                                  x_ext[k, :, q * step:(q + 1) * step])

                # channel-sum (identity-matmul accumulation over the 32 channels)
                pcs = pcsp.tile([128, PFC], f32, tag="pcs")
                for c in range(32):
                    nc.tensor.matmul(pcs[:], identb[:], x_k[:, c * PFC:(c + 1) * PFC],
                                     start=(c == 0), stop=(c == 31),
                                     skip_group_check=True)

                # per-channel spatial sums over own planes -> psum_sp (accumulates)
                off, end = (128, 256) if k == 0 else ((0, 128) if k == 4 else (0, 256))
                n = end - off
                for c in range(32):
                    nc.tensor.matmul(psum_sp[:, off:end], oh[:, c * 32:(c + 1) * 32],
                                     x_k[:, c * PFC + off: c * PFC + end],
                                     start=sp_first[0],
                                     stop=(k == 4 and c == 31),
                                     skip_group_check=True)
                    sp_first[0] = False

                if k == 4:
                    # per-channel spatial sums are complete once this chunk's
                    # sp matmuls land; reduce + ship to DRAM as early as we can
                    nc.vector.tensor_reduce(spsum_col[:], psum_sp[:], axis=Ax.X,
                                            op=Alu.add)
                    nc.gpsimd.dma_start(cc_in[0:1, :], spsum_col[:])
                # per-channel spatial max over own planes: one strided reduce
                # (emitted first: the ca stats gate the whole output phase)
                nc.vector.tensor_reduce(
                    spmax_parts[:, k * 32:(k + 1) * 32],
                    x_k[:].rearrange("p (c f) -> p c f", c=32)[:, :, off:end],
                    axis=Ax.X, op=Alu.max)

                # channel-max: binary tensor_max tree (fp16 runs at 2x mode)
                t1 = treep.tile([128, 4096], bf16, tag="tr1")
                t2 = treep.tile([128, 2048], bf16, tag="tr2")
                t3 = treep.tile([128, 1024], bf16, tag="tr3")
                t4 = treep.tile([128, 512], bf16, tag="tr4")
                cmx = small.tile([128, PFC], bf16, tag="cmx")
                xv = x_k[:].rearrange("p (c f) -> p c f", c=32)
                nc.vector.tensor_max(t1[:].rearrange("p (c f) -> p c f", c=16),
                                     xv[:, 0:32:2, :], xv[:, 1:32:2, :])
                v1 = t1[:].rearrange("p (c f) -> p c f", c=16)
                nc.vector.tensor_max(t2[:].rearrange("p (c f) -> p c f", c=8),
                                     v1[:, 0:16:2, :], v1[:, 1:16:2, :])
                v2 = t2[:].rearrange("p (c f) -> p c f", c=8)
                nc.vector.tensor_max(t3[:].rearrange("p (c f) -> p c f", c=4),
                                     v2[:, 0:8:2, :], v2[:, 1:8:2, :])
                v3 = t3[:].rearrange("p (c f) -> p c f", c=4)
                nc.vector.tensor_max(t4[:].rearrange("p (c f) -> p c f", c=2),
                                     v3[:, 0:4:2, :], v3[:, 1:4:2, :])
                nc.vector.tensor_max(cmx[:], t4[:, 0:256], t4[:, 256:512])

                # ---- s_conv assembly for this chunk's 8 planes ----
                src_av = pcs[:].rearrange("p (d hh) -> p d hh", d=CP)
                dst = s_conv[:].rearrange("p (d h) -> p d h", d=DL)[:, k * CP:(k + 1) * CP, :]
                nc.scalar.activation(
                    dst[0:64, :, 3:67:2], src_av[0:64], Act.Copy, scale=1.0 / 32.0)
                tmp_av = small.tile([128, PFC], bf16, tag="tmpav")
                nc.scalar.activation(tmp_av[64:128, :], pcs[64:128, :], Act.Copy,
                                     scale=1.0 / 32.0)
                sh1 = shiftp.tile([128, PFC], bf16, tag="sh1")
                nc.gpsimd.dma_start(sh1[0:64, :], tmp_av[64:128, :])
                nc.vector.tensor_copy(
                    dst[0:64, :, 4:68:2],
                    sh1[0:64].rearrange("p (d hh) -> p d hh", d=CP))
                nc.vector.tensor_copy(
                    dst[64:128, :, 4:68:2],
                    cmx[64:128].rearrange("p (d hh) -> p d hh", d=CP))
                sh2 = shiftp.tile([128, PFC], bf16, tag="sh2")
                nc.gpsimd.dma_start(sh2[64:128, :], cmx[0:64, :])
                nc.vector.tensor_copy(
                    dst[64:128, :, 3:67:2],
                    sh2[64:128].rearrange("p (d hh) -> p d hh", d=CP))

            def conv_group(g):
                # outputs own planes d_own in [8g, 8g+8) = local d in [8g+4, 8g+12)
                pc_a = pconvp.tile([128, 512], f32, tag="pconv")
                pc_b = pconvp.tile([128, 512], f32, tag="pconv")
                pc = [pc_a, pc_b]
                sc = s_conv[:].rearrange("p (d h) -> p d h", d=DL)
                for t in range(49):
                    kz, ky = t // 7, t % 7
                    d0 = 8 * g + 4 + kz - 3
                    rhs = sc[:, d0:d0 + 8, ky:ky + 64]
                    for pair in range(2):
                        tt = t * 2 + pair
                        nc.tensor.matmul(pc[pair][:],
                                         convw[:, tt * 128:(tt + 1) * 128], rhs,
                                         start=(t == 0), stop=(t == 48),
                                         skip_group_check=True)
                # relu -> sbuf (high priority: must not queue behind
                # collective-gated ca/output work on ACT/DVE)
                hp = tc.high_priority()
                hp.__enter__()
                for pair in range(2):
                    r = relup.tile([128, 512], bf16, tag="relu")
                    nc.scalar.activation(r[:], pc[pair][:], Act.Relu)
                    relu_tiles[g][pair] = r
                # conv2 (1x1x1, 4 -> 1) and sigmoid
                psa = pmiscp.tile([64, 512], f32, tag="m")
                nc.tensor.matmul(psa[:], c2w[:, 0:64], relu_tiles[g][0][:],
                                 start=True, stop=False, skip_group_check=True)
                nc.tensor.matmul(psa[:], c2w[:, 64:128], relu_tiles[g][1][:],
                                 start=False, stop=True, skip_group_check=True)
                sa_w = sawp.tile([64, 512], f32, tag="saw")
                nc.vector.tensor_copy(sa_w[:], psa[:])
                # transpose [64,128] blocks -> sa128, sigmoid fused in the copy
                for b4 in range(4):
                    pt = ptpp.tile([128, 64], f32, tag="ptp")
                    nc.tensor.transpose(pt[:], sa_w[:, b4 * 128:(b4 + 1) * 128],
                                        ident[0:64, 0:64])
                    col = (4 * g + b4) * 64
                    nc.scalar.activation(sa128[:, col:col + 64], pt[:], Act.Sigmoid)
                hp.__exit__(None, None, None)

            def ca_pre():
                # spatial-max: combine chunk partials -> [128, 32] (f32)
                smx = statp.tile([128, 32], f32)
                nc.vector.tensor_reduce(
                    smx[:], spmax_parts[:].rearrange("p (k c) -> p c k", k=NCHUNK),
                    axis=Ax.X, op=Alu.max)
                # partition-axis max on GpSimd (no PE involvement)
                from concourse import bass_isa
                smx_ar = statp.tile([128, 32], f32)
                nc.gpsimd.partition_all_reduce(smx_ar[:], smx[:], 128,
                                               bass_isa.ReduceOp.max)
                spmax_row = smx_ar[0:1, :]
                nc.gpsimd.dma_start(cc_in[1:2, :], spmax_row)
                nc.gpsimd.collective_compute(
                    "AllGather", mybir.AluOpType.bypass,
                    replica_groups=[list(range(NCORES))],
                    ins=[cc_in[:].opt()], outs=[cc_out[:].opt()])
                gath = statp.tile([16, 32], f32)
                nc.gpsimd.dma_start(gath[:], cc_out[:])
                return gath

            def ca_post(gath):
                from concourse import bass_isa
                # rank-dependent pair-combine: mask-multiply + partition reduce
                tS = statp.tile([16, 32], f32)
                nc.vector.tensor_scalar_mul(tS[:], gath[:], masks[:, 0:1])
                tSa = statp.tile([16, 32], f32)
                nc.gpsimd.partition_all_reduce(tSa[:], tS[:], 16,
                                               bass_isa.ReduceOp.add)
                tM = statp.tile([16, 32], f32)
                nc.vector.tensor_scalar_mul(tM[:], gath[:], masks[:, 1:2])
                tMa = statp.tile([16, 32], f32)
                nc.gpsimd.partition_all_reduce(tMa[:], tM[:], 16,
                                               bass_isa.ReduceOp.max)
                hin = statp.tile([1, 64], f32)
                nc.vector.tensor_copy(hin[:, 0:32], tSa[0:1, :])
                nc.vector.tensor_copy(hin[:, 32:64], tMa[0:1, :])
                # MLP via broadcast + fused mul-accumulate (no TensorE)
                hinb = statp.tile([128, 64], f32)
                nc.gpsimd.partition_broadcast(hinb[:], hin[:])
                junk1 = statp.tile([128, 64], f32)
                h1 = statp.tile([128, 1], f32)
                nc.vector.scalar_tensor_tensor(junk1[:], fc1w[:], 1.0, hinb[:],
                                               op0=Alu.bypass, op1=Alu.mult,
                                               accum_out=h1[:])
                hrelu = statp.tile([128, 1], f32)
                nc.vector.tensor_scalar(hrelu[:], h1[:], fc1b[:], 0.0,
                                        op0=Alu.add, op1=Alu.max)
                hrow = statp.tile([1, 128], f32)
                nc.gpsimd.dma_start(hrow[:], hrelu[:])
                hb = statp.tile([32, 128], f32)
                nc.gpsimd.partition_broadcast(hb[:], hrow[:])
                junk2 = statp.tile([32, 128], f32)
                ca0 = statp.tile([32, 1], f32)
                nc.vector.scalar_tensor_tensor(junk2[:], fc2w[:], 1.0, hb[:],
                                               op0=Alu.bypass, op1=Alu.mult,
                                               accum_out=ca0[:])
                ca_col = statp.tile([32, 1], f32)
                nc.scalar.activation(ca_col[:], ca0[:], Act.Sigmoid, bias=fc2b[:])
                ca_row = statp.tile([1, 32], f32)
                nc.gpsimd.dma_start(ca_row[:], ca_col[:])
                nc.gpsimd.partition_broadcast(ca_rep[:], ca_row[:])

            def output_quarter(g, dve_all):
                # outputs for d_own in [8g, 8g+8): sa128 cols [g*256, (g+1)*256)
                sl_sa = slice(g * 256, (g + 1) * 256)
                sa_b4 = sa128[:, sl_sa].rearrange("p (o f) -> p o f", o=1)\
                    .to_broadcast((128, 4, 256))
                for cg in range(8):
                    abuf = outp.tile([128, 1024], bf16, tag="abuf")
                    bbuf = outp.tile([128, 1024], bf16, tag="bbuf")
                    if cg < 4 and not dve_all:
                        for c4 in range(4):
                            c = cg * 4 + c4
                            sl = slice(c4 * 256, (c4 + 1) * 256)
                            nc.scalar.activation(abuf[:, sl], sa128[:, sl_sa],
                                                 Act.Copy, scale=ca_rep[:, c:c + 1])
                    else:
                        # one tensor_tensor covering 4 channels via broadcast APs
                        ca4 = ca_rep[:, cg * 4:(cg + 1) * 4].to_broadcast(
                            (128, 4, 256))
                        nc.vector.tensor_tensor(
                            abuf[:].rearrange("p (c f) -> p c f", c=4),
                            sa_b4, ca4, op=Alu.mult)
                    nc.vector.tensor_scalar(bbuf[:], abuf[:], -1.0, 1.0,
                                            op0=Alu.mult, op1=Alu.add)
                    nc.scalar.dma_start(attn_ext[g, cg], abuf[:])
                    nc.sync.dma_start(anti_ext[g, cg], bbuf[:])

            # ---- schedule ----
            for k in range(NCHUNK):
                stage1_chunk(k)
            gath = ca_pre()
            conv_group(0)
            conv_group(1)
            conv_group(2)
            conv_group(3)
            ca_post(gath)
            output_quarter(0, dve_all=True)
            output_quarter(1, dve_all=True)
            output_quarter(2, dve_all=False)
            output_quarter(3, dve_all=False)

    nc.compile()
    return nc


def _host_inputs(x, fc1_w, fc1_b, fc2_w, fc2_b, conv1_w, conv2_w):
    """Build the per-core input maps (all host-side numpy)."""
    x = np.asarray(x, dtype=np.float32)
    # conv1 Toeplitz lhsT blocks: T[t2][(i,w_in), (o2,w_out)]
    w1 = np.asarray(conv1_w, dtype=np.float32)  # [4, 2, 7, 7, 7]
    T = np.zeros((98, 128, 128), np.float32)
    for kz in range(7):
        for ky in range(7):
            t = kz * 7 + ky
            for pair in range(2):
                t2 = t * 2 + pair
                for o2 in range(2):
                    oc = pair * 2 + o2
                    for i in range(2):
                        for dk in range(7):
                            off = dk - 3  # w_in = w_out + off
                            wv = w1[oc, i, kz, ky, dk]
                            if off >= 0:
                                wo = np.arange(0, 64 - off)
                            else:
                                wo = np.arange(-off, 64)
                            T[t2, i * 64 + wo + off, o2 * 64 + wo] = wv
    convw = np.ascontiguousarray(T.transpose(1, 0, 2).reshape(128, 98 * 128)).astype(BF16)

    oh = np.zeros((128, 32 * 32), BF16)
    for c in range(32):
        oh[:, c * 32 + c] = 1.0
    ident = np.eye(128, dtype=np.float32)
    identb = np.eye(128, dtype=np.float32).astype(BF16)

    c2v = np.asarray(conv2_w, dtype=np.float32).reshape(4)
    c2 = np.zeros((128, 128), np.float32)
    for pair in range(2):
        for o2 in range(2):
            w = np.arange(64)
            c2[o2 * 64 + w, pair * 64 + w] = c2v[pair * 2 + o2]
    c2 = c2.astype(BF16)

    fc1_w = np.asarray(fc1_w, np.float32)           # [128, 64]
    fc1s = fc1_w.copy()
    fc1s[:, 0:32] *= 1.0 / NVOX
    fc1bv = np.asarray(fc1_b, np.float32).reshape(128, 1)
    fc2v = np.ascontiguousarray(np.asarray(fc2_w, np.float32))     # [32, 128]
    fc2bv = np.asarray(fc2_b, np.float32).reshape(32, 1)

    in_maps = []
    for r in range(NCORES):
        b, dhalf = r // 2, r % 2
        xp = np.zeros((C, DL, H, W), np.float32)
        if dhalf == 0:
            xp[:, 4:40] = x[b, :, 0:36]
        else:
            xp[:, 0:36] = x[b, :, 28:64]
        # [c, k, dl, hh, h2, w] -> [k, h2, w, c, dl, hh] -> [5, 128, 8192]
        xr = xp.reshape(C, NCHUNK, CP, 32, 2, W).transpose(1, 4, 5, 0, 2, 3)
        xhost = np.ascontiguousarray(xr.reshape(NCHUNK, 128, 32 * PFC)).astype(BF16)

        partner = r ^ 1
        masks = np.zeros((16, 2), np.float32)
        masks[2 * r, 0] = 1.0
        masks[2 * partner, 0] = 1.0
        masks[2 * r + 1, 1] = 1.0
        masks[2 * partner + 1, 1] = 1.0

        in_maps.append({
            "x": xhost, "convw": convw, "oh": oh, "ident": ident, "identb": identb, "c2w": c2,
            "fc1w": fc1s, "fc1b": fc1bv, "fc2w": fc2v, "fc2b": fc2bv,
            "masks": masks,
        })
    return in_maps


def _decode_out(arr):
    """[4, 8, 128, 1024] -> [C, 32, H, W] (own planes)."""
    a = np.asarray(arr, dtype=np.float32)
    a = a.reshape(4, 8, 2, 64, 4, 4, 64)            # g, cg, d2, h, c4, dl, w
    a = a.transpose(1, 4, 0, 5, 2, 3, 6)            # cg, c4, g, dl, d2, h, w
    return a.reshape(C, 32, H, W)


def _install_ntff_shim():
    """The agent image's antenv lacks axon_hooks; recreate it so
    run_bass_kernel_spmd(trace=True) can NTFF-profile via libaxon."""
    import sys, types, contextlib, ctypes
    try:
        import antenv.axon_hooks  # noqa
        return
    except ImportError:
        pass
    so_path = "/opt/axon/libaxon_pjrt.so"
    lib = ctypes.CDLL(so_path)
    if not hasattr(lib, "axon_start_nrt_profile"):
        return
    lib.axon_start_nrt_profile.argtypes = [ctypes.POINTER(ctypes.c_int64),
                                           ctypes.c_size_t]
    lib.axon_start_nrt_profile.restype = ctypes.c_int64
    lib.axon_stop_nrt_profile.argtypes = [ctypes.c_char_p]
    lib.axon_stop_nrt_profile.restype = ctypes.c_int64

    @contextlib.contextmanager
    def _hook(output_dir, device_ids):
        import jax
        jax.devices()
        if device_ids:
            ids = (ctypes.c_int64 * len(device_ids))(*device_ids)
            rc = lib.axon_start_nrt_profile(ids, len(device_ids))
        else:
            rc = lib.axon_start_nrt_profile(None, 0)
        if rc != 0:
            raise RuntimeError(f"axon_start_nrt_profile rc={rc}")
        try:
            yield
        finally:
            n = lib.axon_stop_nrt_profile(str(output_dir).encode())
            print(f"profile: {n} file(s) written to {output_dir}")

    mod = types.ModuleType("antenv.axon_hooks")
    _state = {"hook": _hook}
    mod.get_axon_ntff_profile_hook = lambda: _state["hook"]
    mod.set_axon_ntff_profile_hook = lambda h: _state.__setitem__("hook", h)
    sys.modules["antenv.axon_hooks"] = mod


def kernel(x, fc1_w, fc1_b, fc2_w, fc2_b, conv1_w, conv2_w, _want_time=False):
    from concourse.bass_utils import run_bass_kernel_spmd
    if _want_time:
        _install_ntff_shim()

    if "nc" not in _CACHE:
        _CACHE["nc"] = _build_nc()
    nc = _CACHE["nc"]

    in_maps = _host_inputs(x, fc1_w, fc1_b, fc2_w, fc2_b, conv1_w, conv2_w)
    res = run_bass_kernel_spmd(nc, in_maps, core_ids=list(range(NCORES)),
                               trace=bool(_want_time))
    attention = np.empty((B, C, D, H, W), np.float32)
    anti = np.empty((B, C, D, H, W), np.float32)
    for r in range(NCORES):
        b, dhalf = r // 2, r % 2
        d0 = dhalf * 32
        attention[b, :, d0:d0 + 32] = _decode_out(res.results[r]["attn"])
        anti[b, :, d0:d0 + 32] = _decode_out(res.results[r]["anti"])
    if _want_time:
        return (attention, anti), res.exec_time_ns
    return attention, anti
